# revision 1
# baseline (speedup 1.0000x reference)
"""Trainium2 Bass kernel for nn_LinearAttention (gated linear attention).

Math (per reference):
    qkv = x @ Wqkv.T ; q,k,v = split(qkv); q,k = elu(.)+1
    per (b,h): running_kv[t]  = d*running_kv[t-1]  + k[t]*v[t]   (elementwise, D=64)
               running_ksum[t]= d*running_ksum[t-1]+ k[t]
    den = clip(sum_d(q*running_ksum), 1e-6); out = q*running_kv/den
    g = sigmoid(out @ Wgate.T + bgate); out = g*out + (1-g)*v
    y = out @ Wout.T

Implementation strategy (8 NeuronCores, SPMD, no collectives):
  - Token-parallel: core c handles batch b=c//2, T-half h=c%2 (2048 tokens)
    plus a 512-token halo before the chunk to warm the decay scan
    (decay=0.95 => truncation error ~0.95^512 ~ 4e-12).  Half 0 gets a
    zero halo + k-mask so its scan state is exactly 0 at t=0.
  - Everything on-chip lives as [feature(partition), token(free)]; the host
    pre-transposes x and the weight matrices so both matmul operands are in
    natural layout and no on-chip transpose is ever needed.  The final
    output is produced transposed ([hidden, T]) and un-transposed on host.
  - The decay scan runs natively on the Vector engine via
    tensor_tensor_scan (state = d*state + u along the free/time axis),
    chained across 512-token groups via initial=prev[:, -1:].
  - den: sum over D=64 partitions via a 0/1 block-diagonal selector matmul
    (PSUM [16,512]); reciprocal broadcast back to 128 partitions via a
    second selector matmul in fp32r.
  - phi(x)=elu(x)+1 = exp(min(x,0)) + relu(x): DVE min, ACT Exp, then one
    fused scalar_tensor_tensor (max 0 then add).
  - bgate rides the Sigmoid drain as the ACT per-partition bias.
"""

import sys

for _p in ('/opt/trn_rl_repo', '/root/.axon_site'):
    if _p not in sys.path:
        sys.path.insert(0, _p)

from contextlib import ExitStack

import ml_dtypes
import numpy as np

import concourse.tile as tile
from concourse import bacc, mybir
from concourse.bass_utils import run_bass_kernel_spmd

F32 = mybir.dt.float32
BF16 = mybir.dt.bfloat16
AL = mybir.AluOpType
AF = mybir.ActivationFunctionType

B, T, HID = 4, 4096, 1024
H, D = 16, 64
OD = 3 * HID              # 3072 qkv output rows
NK = HID // 128           # 8 hidden (contraction) tiles
NOT = OD // 128           # 24 od tiles: q=0..7, k=8..15, v=16..23
HALF_T = T // 2           # 2048 tokens per core
HALO = 512
TLOC = HALO + HALF_T      # 2560
WG = 512                  # token-group width
NG = TLOC // WG           # 5 groups; group 0 = halo
NH = HID // 128           # 8 tiles per q/k/v section

_cache = {}


def _build_nc():
    nc = bacc.Bacc("TRN2", target_bir_lowering=False, debug=False)

    xT = nc.dram_tensor("xT", [HID, TLOC], BF16, kind="ExternalInput")
    wqkvT = nc.dram_tensor("wqkvT", [HID, OD], BF16, kind="ExternalInput")
    wgateT = nc.dram_tensor("wgateT", [HID, HID], BF16, kind="ExternalInput")
    woutT = nc.dram_tensor("woutT", [HID, HID], BF16, kind="ExternalInput")
    dec_c = nc.dram_tensor("dec_c", [128, NH], F32, kind="ExternalInput")
    mask_c = nc.dram_tensor("mask_c", [128, 1], F32, kind="ExternalInput")
    densel = nc.dram_tensor("densel", [128, NH * H], BF16, kind="ExternalInput")
    bcsel = nc.dram_tensor("bcsel", [H, NH * 128], mybir.dt.float32r,
                           kind="ExternalInput")
    bgate_c = nc.dram_tensor("bgate_c", [128, NH], F32, kind="ExternalInput")
    yT = nc.dram_tensor("yT", [HID, HALF_T], F32, kind="ExternalOutput")

    with tile.TileContext(nc) as tc, ExitStack() as ctx:
        consts = ctx.enter_context(tc.tile_pool(name="consts", bufs=1))
        wq_pool = ctx.enter_context(tc.tile_pool(name="wq", bufs=1))
        wg_pool = ctx.enter_context(tc.tile_pool(name="wgp", bufs=1))
        wo_pool = ctx.enter_context(tc.tile_pool(name="wop", bufs=1))
        xt_pool = ctx.enter_context(tc.tile_pool(name="xt", bufs=16))
        qkv_pool = ctx.enter_context(tc.tile_pool(name="qkv", bufs=9))
        tmp_pool = ctx.enter_context(tc.tile_pool(name="tmp", bufs=2))
        cum_pool = ctx.enter_context(tc.tile_pool(name="cum", bufs=1))
        st_pool = ctx.enter_context(tc.tile_pool(name="st", bufs=2))
        oa_pool = ctx.enter_context(tc.tile_pool(name="oa", bufs=9))
        gt_pool = ctx.enter_context(tc.tile_pool(name="gt", bufs=2))
        mix_pool = ctx.enter_context(tc.tile_pool(name="mix", bufs=9))
        y_pool = ctx.enter_context(tc.tile_pool(name="ysb", bufs=2))
        ps_pool = ctx.enter_context(tc.tile_pool(name="ps", bufs=7, space="PSUM"))
        psd_pool = ctx.enter_context(tc.tile_pool(name="psd", bufs=1, space="PSUM"))

        # small consts first (mask gates the halo k drains)
        dec_s = consts.tile([128, NH], F32, tag="dec")
        nc.gpsimd.dma_start(dec_s[:], dec_c.ap()[:, :])
        mask_s = consts.tile([128, 1], F32, tag="mask")
        nc.gpsimd.dma_start(mask_s[:], mask_c.ap()[:, :])

        # qkv weights: one SBUF tile per hid k-tile, loaded by od section in
        # the order the pipeline consumes them: k-sections, then (after the
        # first x tiles, emitted by the pipeline below) v- and q-sections.
        wq_sec = {}
        for sec in range(3):
            wq_sec[sec] = []
            for k in range(NK):
                w = wq_pool.tile([128, HID], BF16, tag=f"wq{sec}_{k}",
                                 name=f"wq_{sec}_{k}")
                wq_sec[sec].append(w)

        def load_wq_sec(sec, eng=None):
            eng = eng or nc.sync
            for k in range(NK):
                eng.dma_start(
                    wq_sec[sec][k][:],
                    wqkvT.ap()[128 * k:128 * (k + 1), HID * sec:HID * (sec + 1)])

        load_wq_sec(1)  # k-section: first thing the PE needs

        densel_s = consts.tile([128, NH * H], BF16, tag="densel")
        bcsel_s = consts.tile([H, NH * 128], mybir.dt.float32r, tag="bcsel")
        bgate_s = consts.tile([128, NH], F32, tag="bg")
        wg_s, wo_s = [], []
        for k in range(NK):
            wg_s.append(wg_pool.tile([128, HID], BF16, tag=f"wg{k}",
                                     name=f"wg_{k}"))
            wo_s.append(wo_pool.tile([128, HID], BF16, tag=f"wo{k}",
                                     name=f"wo_{k}"))

        def load_rest():
            nc.gpsimd.dma_start(densel_s[:], densel.ap()[:, :])
            nc.gpsimd.dma_start(bcsel_s[:], bcsel.ap()[:, :])
            nc.gpsimd.dma_start(bgate_s[:], bgate_c.ap()[:, :])
            for k in range(NK):
                nc.gpsimd.dma_start(
                    wg_s[k][:], wgateT.ap()[128 * k:128 * (k + 1), :])
                nc.gpsimd.dma_start(
                    wo_s[k][:], woutT.ap()[128 * k:128 * (k + 1), :])

        state = {}

        def emit_xt(g):
            tok = slice(g * WG, (g + 1) * WG)
            xts = []
            for k in range(NK):
                xt_t = xt_pool.tile([128, WG], BF16, tag="xt", name=f"xt_{g}_{k}")
                nc.sync.dma_start(xt_t[:], xT.ap()[128 * k:128 * (k + 1), tok])
                xts.append(xt_t)
            return xts

        def emit_qkv(g, xts, ots, q1, k1, vv):
            """PE: qkv matmuls for the given od tiles; DVE/ACT: phi drains."""
            is_halo = g == 0
            for ot in ots:
                sec, oti = divmod(ot, NH)
                ps = ps_pool.tile([128, WG], F32, tag="mm", name=f"qkvp_{g}_{ot}")
                for k in range(NK):
                    nc.tensor.matmul(
                        ps[:], wq_sec[sec][k][:, 128 * oti:128 * (oti + 1)],
                        xts[k][:], start=(k == 0), stop=(k == NK - 1))
                if ot < 2 * NH:  # q or k: phi drain via single psum copy
                    qc = tmp_pool.tile([128, WG], BF16, tag="qc", bufs=2,
                                       name=f"qc_{g}_{ot}")
                    nc.scalar.copy(qc[:], ps[:])
                    qm = tmp_pool.tile([128, WG], BF16, tag="phim", bufs=2,
                                       name=f"qm_{g}_{ot}")
                    nc.vector.tensor_scalar_min(qm[:], qc[:], 0.0)
                    qe = tmp_pool.tile([128, WG], BF16, tag="phie", bufs=2,
                                       name=f"qe_{g}_{ot}")
                    nc.scalar.activation(qe[:], qm[:], AF.Exp)
                    if ot < NH:
                        j = ot
                        q1[j] = qkv_pool.tile([128, WG], BF16, tag="q1",
                                              name=f"q1_{g}_{j}")
                        nc.vector.scalar_tensor_tensor(
                            q1[j][:], qc[:], 0.0, qe[:], AL.max, AL.add)
                    elif is_halo:
                        j = ot - NH
                        kr = tmp_pool.tile([128, WG], BF16, tag="kraw", bufs=1,
                                           name=f"kr_{g}_{j}")
                        nc.vector.scalar_tensor_tensor(
                            kr[:], qc[:], 0.0, qe[:], AL.max, AL.add)
                        k1[j] = qkv_pool.tile([128, WG], BF16, tag="k1",
                                              name=f"k1_{g}_{j}")
                        nc.vector.tensor_scalar_mul(
                            k1[j][:], kr[:], mask_s[:, 0:1])
                    else:
                        j = ot - NH
                        k1[j] = qkv_pool.tile([128, WG], BF16, tag="k1",
                                              name=f"k1_{g}_{j}")
                        nc.vector.scalar_tensor_tensor(
                            k1[j][:], qc[:], 0.0, qe[:], AL.max, AL.add)
                else:  # v
                    j = ot - 2 * NH
                    vv[j] = qkv_pool.tile([128, WG], BF16, tag="v", bufs=9,
                                          name=f"v_{g}_{j}")
                    nc.scalar.copy(vv[j][:], ps[:])

        def emit_oa_dl(g, q1, cum_kv, den_i, vv):
            """qckv mults, bc broadcast matmuls, attention out, and the
            (oa - v) delta — after which v is dead."""
            qckv = [None] * NH
            for j in range(NH):
                qckv[j] = tmp_pool.tile([128, WG], BF16, tag="qckv", bufs=2,
                                        name=f"qckv_{g}_{j}")
                nc.vector.tensor_mul(qckv[j][:], q1[j][:], cum_kv[j][:])
            oa = [None] * NH
            dls = [None] * NH
            for j in range(NH):
                bc = ps_pool.tile([128, WG], F32, tag="mm", name=f"bc_{g}_{j}")
                nc.tensor.matmul(
                    bc[:], bcsel_s[:, 128 * j:128 * (j + 1)], den_i[:, :],
                    start=True, stop=True)
                oa[j] = oa_pool.tile([128, WG], BF16, tag="oa",
                                     name=f"oa_{g}_{j}")
                nc.vector.tensor_mul(oa[j][:], qckv[j][:], bc[:])
                dls[j] = tmp_pool.tile([128, WG], BF16, tag="dl", bufs=9,
                                       name=f"dl_{g}_{j}")
                nc.gpsimd.tensor_sub(dls[j][:], oa[j][:], vv[j][:])
            return oa, dls

        def emit_gate(g, oa):
            gts = [None] * NH
            for ot in range(NH):
                ps = ps_pool.tile([128, WG], F32, tag="mm", name=f"gp_{g}_{ot}")
                for k in range(NK):
                    nc.tensor.matmul(
                        ps[:], wg_s[k][:, 128 * ot:128 * (ot + 1)], oa[k][:],
                        start=(k == 0), stop=(k == NK - 1))
                gts[ot] = gt_pool.tile([128, WG], BF16, tag="gt",
                                       name=f"gt_{g}_{ot}")
                nc.scalar.activation(
                    gts[ot][:], ps[:], AF.Sigmoid, bias=bgate_s[:, ot:ot + 1])
            return gts

        def emit_mix(g, gts, dls, oa):
            # mix = g*oa + (1-g)*v = (g-1)*(oa-v) + oa = (gt-1)*dl + oa
            mix = [None] * NH
            for ot in range(NH):
                d2 = tmp_pool.tile([128, WG], BF16, tag="gd",
                                   name=f"d2_{g}_{ot}")
                nc.vector.scalar_tensor_tensor(
                    d2[:], gts[ot][:], -1.0, dls[ot][:], AL.add, AL.mult)
                mix[ot] = mix_pool.tile([128, WG], BF16, tag="mix",
                                        name=f"mix_{g}_{ot}")
                nc.vector.tensor_add(mix[ot][:], d2[:], oa[ot][:])
            return mix

        def emit_y(g, mix):
            out_tok = slice(g * WG - HALO, g * WG - HALO + WG)
            for ot in range(NH):
                ps = ps_pool.tile([128, WG], F32, tag="mm", name=f"yp_{g}_{ot}")
                for k in range(NK):
                    nc.tensor.matmul(
                        ps[:], wo_s[k][:, 128 * ot:128 * (ot + 1)], mix[k][:],
                        start=(k == 0), stop=(k == NK - 1))
                ysb = y_pool.tile([128, WG], F32, tag="ysb",
                                  name=f"ysb_{g}_{ot}")
                nc.scalar.copy(ysb[:], ps[:])
                nc.sync.dma_start(
                    yT.ap()[128 * ot:128 * (ot + 1), out_tok], ysb[:])

        def emit_ksum_scans(g, k1, q1):
            """ksum scans + prod tiles: emitted right after the q-section so
            the den chain completes early in the iteration."""
            cum_ks = [None] * NH
            for j in range(NH):
                dec_b = dec_s[:, j:j + 1].broadcast_to([128, WG])
                cum_ks[j] = cum_pool.tile([128, WG], BF16, tag=f"cks{j}",
                                          name=f"cks_{g}_{j}")
                init_ks = 0.0 if g == 0 else state["ks"][j][:, 0:1]
                nc.vector.tensor_tensor_scan(
                    cum_ks[j][:], dec_b, k1[j][:], init_ks, AL.mult, AL.add)
            prods = [None] * NH
            if q1[0] is not None:
                for j in range(NH):
                    prods[j] = tmp_pool.tile([128, WG], BF16, tag="prod",
                                             bufs=9, name=f"prod_{g}_{j}")
                    nc.vector.tensor_mul(prods[j][:], q1[j][:], cum_ks[j][:])
            nks = [None] * NH
            if g < NG - 1:
                for j in range(NH):
                    nks[j] = st_pool.tile([128, 1], F32, tag=f"sks{j}",
                                          name=f"sks_{g}_{j}")
                    nc.gpsimd.tensor_copy(nks[j][:], cum_ks[j][:, WG - 1:WG])
            state["ks"] = nks
            return cum_ks, prods

        def emit_kv_scans(g, k1, vv):
            cum_kv = [None] * NH
            kvs = [None] * NH
            for j in range(NH):
                kvs[j] = tmp_pool.tile([128, WG], BF16, tag="kvp", bufs=2,
                                       name=f"kv_{g}_{j}")
                nc.gpsimd.tensor_mul(kvs[j][:], k1[j][:], vv[j][:])
            for j in range(NH):
                dec_b = dec_s[:, j:j + 1].broadcast_to([128, WG])
                cum_kv[j] = cum_pool.tile([128, WG], BF16, tag=f"ckv{j}",
                                          name=f"ckv_{g}_{j}")
                init_kv = 0.0 if g == 0 else state["kv"][j][:, 0:1]
                nc.vector.tensor_tensor_scan(
                    cum_kv[j][:], dec_b, kvs[j][:], init_kv, AL.mult, AL.add)
            nkv = [None] * NH
            if g < NG - 1:
                for j in range(NH):
                    nkv[j] = st_pool.tile([128, 1], F32, tag=f"skv{j}",
                                          name=f"skv_{g}_{j}")
                    nc.gpsimd.tensor_copy(nkv[j][:], cum_kv[j][:, WG - 1:WG])
            state["kv"] = nkv
            return cum_kv

        def emit_den(g, prods):
            dps = psd_pool.tile([H, WG], F32, tag="den", name=f"dps_{g}")
            for j in range(NH):
                nc.tensor.matmul(
                    dps[:], densel_s[:, H * j:H * (j + 1)], prods[j][:],
                    start=(j == 0), stop=(j == NH - 1))
            den_r = tmp_pool.tile([H, WG], F32, tag="denr", name=f"denr_{g}")
            nc.vector.tensor_scalar_max(den_r[:], dps[:], 1e-6)
            den_i = tmp_pool.tile([H, WG], mybir.dt.float32r, tag="deni",
                                  name=f"deni_{g}")
            with nc.allow_low_precision(reason="fp32r broadcast of reciprocal"):
                nc.vector.reciprocal(den_i[:], den_r[:])
            return den_i

        # ---- software-pipelined emission --------------------------------
        # iter g: [xt][oa/dl g-1][q g][k g][v g][gate g-1][scans g]
        #         [mix g-1][den g][y g-1]
        # The den chain for group g completes a full iteration before its
        # bc-matmul consumer; v dies at the dl subtraction so tile live
        # sets fit their pools.
        k_sec = list(range(NH, 2 * NH))
        q_sec = list(range(NH))
        v_sec = list(range(2 * NH, NOT))
        prev = None
        for g in range(NG):
            q1 = [None] * NH
            k1 = [None] * NH
            vv = [None] * NH
            xts = emit_xt(g)
            if g == 0:
                load_wq_sec(2)  # v-section, after xt g0 in queue order
                load_wq_sec(0)  # q-section next (needed ~45us in)
            emit_qkv(g, xts, k_sec, q1, k1, vv)
            if prev is not None:
                p_q1, p_ckv, p_vv, p_den, pg = prev
                oa, dls = emit_oa_dl(pg, p_q1, p_ckv, p_den, p_vv)
            if g > 0:
                emit_qkv(g, xts, q_sec, q1, k1, vv)
            cum_ks, prods = emit_ksum_scans(g, k1, q1)
            emit_qkv(g, xts, v_sec, q1, k1, vv)
            if g == 1:
                load_rest()
            if g > 0:
                den_i = emit_den(g, prods)
            if prev is not None:
                gts = emit_gate(pg, oa)
            cum_kv = emit_kv_scans(g, k1, vv)
            if prev is not None:
                mix = emit_mix(pg, gts, dls, oa)
                emit_y(pg, mix)
            if g > 0:
                prev = (q1, cum_kv, vv, den_i, g)
        q1, cum_kv, vv, den_i, g = prev
        oa, dls = emit_oa_dl(g, q1, cum_kv, den_i, vv)
        gts = emit_gate(g, oa)
        mix = emit_mix(g, gts, dls, oa)
        emit_y(g, mix)

    nc.compile()
    return nc


def _sigmoid(v):
    return 1.0 / (1.0 + np.exp(-v))


def _make_inputs(x, Wqkv, Wout, Wgate, bgate, decay_param):
    decay = _sigmoid(np.asarray(decay_param, np.float64)).astype(np.float32)
    bf = ml_dtypes.bfloat16
    wqkvT = np.ascontiguousarray(np.asarray(Wqkv, np.float32).T).astype(bf)
    wgateT = np.ascontiguousarray(np.asarray(Wgate, np.float32).T).astype(bf)
    woutT = np.ascontiguousarray(np.asarray(Wout, np.float32).T).astype(bf)

    p = np.arange(128)
    dec_c = np.empty((128, NH), np.float32)
    for j in range(NH):
        dec_c[:, j] = decay[2 * j + p // 64]
    densel = np.zeros((128, NH * H), np.float32)
    for j in range(NH):
        for pp in range(128):
            densel[pp, H * j + 2 * j + pp // 64] = 1.0
    bcsel = np.zeros((H, NH * 128), np.float32)
    for j in range(NH):
        for m in range(128):
            bcsel[2 * j + m // 64, 128 * j + m] = 1.0
    bgate_c = np.ascontiguousarray(
        np.asarray(bgate, np.float32).reshape(NH, 128).T)

    in_maps = []
    for c in range(8):
        b, half = c // 2, c % 2
        xb = np.asarray(x[b], np.float32)  # [T, HID]
        if half == 0:
            xloc = np.concatenate(
                [np.zeros((HALO, HID), np.float32), xb[:HALF_T]], axis=0)
            mask = np.zeros((128, 1), np.float32)
        else:
            xloc = xb[HALF_T - HALO:]
            mask = np.ones((128, 1), np.float32)
        in_maps.append({
            "xT": np.ascontiguousarray(xloc.T).astype(bf),
            "wqkvT": wqkvT, "wgateT": wgateT, "woutT": woutT,
            "dec_c": dec_c, "mask_c": mask,
            "densel": densel.astype(bf), "bcsel": bcsel,
            "bgate_c": bgate_c,
        })
    return in_maps


def kernel(x, Wqkv, Wout, Wgate, bgate, decay_param):
    if "nc" not in _cache:
        _cache["nc"] = _build_nc()
    nc = _cache["nc"]
    in_maps = _make_inputs(x, Wqkv, Wout, Wgate, bgate, decay_param)
    res = run_bass_kernel_spmd(nc, in_maps, list(range(8)))
    y = np.empty((B, T, HID), np.float32)
    for c in range(8):
        b, half = c // 2, c % 2
        y[b, half * HALF_T:(half + 1) * HALF_T, :] = res.results[c]["yT"].T
    return y



# revision 13
# speedup vs baseline: 1.0832x; 1.0832x over previous
"""Trainium2 Bass kernel for nn_LinearAttention (gated linear attention).

Math (per reference):
    qkv = x @ Wqkv.T ; q,k,v = split(qkv); q,k = elu(.)+1
    per (b,h): running_kv[t]  = d*running_kv[t-1]  + k[t]*v[t]   (elementwise, D=64)
               running_ksum[t]= d*running_ksum[t-1]+ k[t]
    den = clip(sum_d(q*running_ksum), 1e-6); out = q*running_kv/den
    g = sigmoid(out @ Wgate.T + bgate); out = g*out + (1-g)*v
    y = out @ Wout.T

Implementation strategy (8 NeuronCores, SPMD, no collectives):
  - Token-parallel: core c handles batch b=c//2, T-half h=c%2 (2048 tokens)
    plus a 128-token halo before the chunk to warm the decay scan
    (decay=0.95 => truncation error ~0.95^128 ~ 1.4e-3 relative, well under
    tolerance).  Half 0 gets a zero halo + k-mask so its state is exactly 0.
  - Layout: [feature(partition), token(free)]; host pre-transposes x and the
    weights so no on-chip transpose is needed; y comes out transposed.
  - Decay scans run on the Vector engine via tensor_tensor_scan, chained
    across token groups (g0=128-halo, then 4x512) via initial=state[:, -1:].
  - phi(x)=elu(x)+1 = min(exp(x),1) + relu(x): ACT Exp straight from PSUM,
    DVE tensor_scalar min (4x mode), DVE scalar_tensor_tensor (relu+add,
    PSUM src).  No PSUM->SBUF staging copy.
  - den: 0/1 block-diagonal selector matmul -> PSUM [16,512];
    reciprocal_approx_fast (custom DVE, ~5x faster than RECIPROCAL), cast to
    f32r, broadcast back to 128 partitions via selector matmul.
  - Steady-state per-engine schedule is software-pipelined so the PE never
    waits on the DVE FIFO:
      PE:  [k0-3 | bc(g-1) | k4-7 | q | gate(g-1) | den | v | y(g-1)]
      DVE: [phi-k | oa | phi-k | ksum-scans | phi-q | prods | recip |
            (g-1 mix) | kv-scans]
      ACT: [exp-k | exp-q | sigmoid | v-copy | y-copy]
      GPS: [ks-states | mix-mul | kvs | qckv | kv-states]
  - ~10 garbage matmuls on a memset tile at t=0 keep the PE HAM clock warm
    while the first weight DMAs land (spread over 4 DMA queues).
"""

import sys

for _p in ('/opt/trn_rl_repo', '/root/.axon_site'):
    if _p not in sys.path:
        sys.path.insert(0, _p)

from contextlib import ExitStack

import ml_dtypes
import numpy as np

import concourse.tile as tile
from concourse import bacc, mybir
from concourse.bass_utils import run_bass_kernel_spmd

F32 = mybir.dt.float32
F32R = mybir.dt.float32r
BF16 = mybir.dt.bfloat16
AL = mybir.AluOpType
AF = mybir.ActivationFunctionType

B, T, HID = 4, 4096, 1024
H, D = 16, 64
OD = 3 * HID              # 3072 qkv output rows
NK = HID // 128           # 8 hidden (contraction) tiles
HALF_T = T // 2           # 2048 tokens per core
HALO = 128
TLOC = HALO + HALF_T      # 2176
NH = HID // 128           # 8 tiles per q/k/v section
WIDTHS = [HALO, 512, 512, 512, 512]
OFFS = [0, 128, 640, 1152, 1664]
NG = len(WIDTHS)

_cache = {}


def _build_nc():
    nc = bacc.Bacc("TRN2", target_bir_lowering=False, debug=False)

    xT = nc.dram_tensor("xT", [HID, TLOC], BF16, kind="ExternalInput")
    wqkvT = nc.dram_tensor("wqkvT", [HID, OD], BF16, kind="ExternalInput")
    wgateT = nc.dram_tensor("wgateT", [HID, HID], BF16, kind="ExternalInput")
    woutT = nc.dram_tensor("woutT", [HID, HID], BF16, kind="ExternalInput")
    dec_c = nc.dram_tensor("dec_c", [128, NH], F32, kind="ExternalInput")
    mask_c = nc.dram_tensor("mask_c", [128, 1], F32, kind="ExternalInput")
    densel = nc.dram_tensor("densel", [128, NH * H], BF16, kind="ExternalInput")
    bcsel = nc.dram_tensor("bcsel", [H, NH * 128], F32R, kind="ExternalInput")
    bgate_c = nc.dram_tensor("bgate_c", [128, NH], F32, kind="ExternalInput")
    yT = nc.dram_tensor("yT", [HID, HALF_T], BF16, kind="ExternalOutput")

    with tile.TileContext(nc) as tc, ExitStack() as ctx:
        consts = ctx.enter_context(tc.tile_pool(name="consts", bufs=1))
        wq_pool = ctx.enter_context(tc.tile_pool(name="wq", bufs=1))
        wg_pool = ctx.enter_context(tc.tile_pool(name="wgp", bufs=1))
        wo_pool = ctx.enter_context(tc.tile_pool(name="wop", bufs=1))
        xt_pool = ctx.enter_context(tc.tile_pool(name="xt", bufs=10))
        qkv_pool = ctx.enter_context(tc.tile_pool(name="qkv", bufs=9))
        tmp_pool = ctx.enter_context(tc.tile_pool(name="tmp", bufs=2))
        cum_pool = ctx.enter_context(tc.tile_pool(name="cum", bufs=1))
        st_pool = ctx.enter_context(tc.tile_pool(name="st", bufs=2))
        oa_pool = ctx.enter_context(tc.tile_pool(name="oa", bufs=8))
        gt_pool = ctx.enter_context(tc.tile_pool(name="gt", bufs=8))
        mix_pool = ctx.enter_context(tc.tile_pool(name="mix", bufs=8))
        y_pool = ctx.enter_context(tc.tile_pool(name="ysb", bufs=3))
        ps_pool = ctx.enter_context(tc.tile_pool(name="ps", bufs=7, space="PSUM"))
        psd_pool = ctx.enter_context(tc.tile_pool(name="psd", bufs=1, space="PSUM"))

        # ---- warmup: keep the PE HAM clock busy while weight DMAs land ----
        warm = consts.tile([128, 256], BF16, tag="warm")
        nc.gpsimd.memset(warm[:], 0.25)
        wps = ps_pool.tile([128, 512], F32, tag="mm", name="warm_ps")
        for i in range(28):
            nc.tensor.matmul(wps[:, 0:256], warm[:, 0:128], warm[:],
                             start=True, stop=True)

        # ---- small consts (gpsimd queue) ----
        dec_s = consts.tile([128, NH], F32, tag="dec")
        nc.gpsimd.dma_start(dec_s[:], dec_c.ap()[:, :])
        mask_s = consts.tile([128, 1], F32, tag="mask")
        nc.gpsimd.dma_start(mask_s[:], mask_c.ap()[:, :])
        densel_s = consts.tile([128, NH * H], BF16, tag="densel")
        nc.gpsimd.dma_start(densel_s[:], densel.ap()[:, :])
        bcsel_s = consts.tile([H, NH * 128], F32R, tag="bcsel")
        nc.gpsimd.dma_start(bcsel_s[:], bcsel.ap()[:, :])
        bgate_s = consts.tile([128, NH], F32, tag="bg")
        nc.gpsimd.dma_start(bgate_s[:], bgate_c.ap()[:, :])

        # ---- qkv weights: interleave k-tiles across the act/vector queues
        # in consumption order: k-section, v-section, q-section ----
        wq_sec = {}
        for sec in range(3):
            wq_sec[sec] = [
                wq_pool.tile([128, HID], BF16, tag=f"wq{sec}_{k}",
                             name=f"wq_{sec}_{k}")
                for k in range(NK)]
        wg_s, wo_s = [], []
        for k in range(NK):
            wg_s.append(wg_pool.tile([128, HID], BF16, tag=f"wg{k}",
                                     name=f"wg_{k}"))
            wo_s.append(wo_pool.tile([128, HID], BF16, tag=f"wo{k}",
                                     name=f"wo_{k}"))

        def load_wq_sec(sec):
            for k in range(NK):
                eng = nc.scalar if k % 2 == 0 else nc.gpsimd
                eng.dma_start(
                    wq_sec[sec][k][:],
                    wqkvT.ap()[128 * k:128 * (k + 1), HID * sec:HID * (sec + 1)])

        # weights alternate between the act and gpsimd DMA rings, in
        # consumption order: k-section, v-section, q-section, gate, out.
        load_wq_sec(1)
        load_wq_sec(2)
        load_wq_sec(0)
        for k in range(NK):
            eng = nc.scalar if k % 2 == 0 else nc.gpsimd
            eng.dma_start(wg_s[k][:], wgateT.ap()[128 * k:128 * (k + 1), :])
        for k in range(NK):
            eng = nc.scalar if k % 2 == 0 else nc.gpsimd
            eng.dma_start(wo_s[k][:], woutT.ap()[128 * k:128 * (k + 1), :])

        state = {}

        def emit_xt(g):
            W = WIDTHS[g]
            tok = slice(OFFS[g], OFFS[g] + W)
            xts = []
            for k in range(NK):
                xt_t = xt_pool.tile([128, W], BF16,
                                    tag="xt0" if g == 0 else "xt",
                                    bufs=8 if g == 0 else 10,
                                    name=f"xt_{g}_{k}")
                nc.sync.dma_start(xt_t[:], xT.ap()[128 * k:128 * (k + 1), tok])
                xts.append(xt_t)
            return xts

        def emit_sec(g, xts, sec, out_list, tag):
            """One qkv section (8 od tiles): PE matmuls + phi/copy drains.
            js selects which od tiles of the section to emit."""
            W = WIDTHS[g]
            is_halo = g == 0
            for j in range(NH):
                ps = ps_pool.tile([128, W], F32, tag="mm",
                                  name=f"p{sec}_{g}_{j}")
                for k in range(NK):
                    nc.tensor.matmul(
                        ps[:], wq_sec[sec][k][:, 128 * j:128 * (j + 1)],
                        xts[k][:], start=(k == 0), stop=(k == NK - 1))
                if sec == 2:  # v: plain copy
                    out_list[j] = qkv_pool.tile(
                        [128, W], BF16, tag="v0" if is_halo else "v",
                        bufs=8 if is_halo else 9,
                        name=f"v_{g}_{j}")
                    nc.scalar.copy(out_list[j][:], ps[:])
                else:
                    qe = tmp_pool.tile([128, W], BF16, tag="qe", bufs=2,
                                       name=f"qe_{sec}_{g}_{j}")
                    nc.scalar.activation(qe[:], ps[:], AF.Exp)
                    em = tmp_pool.tile([128, W], BF16, tag="em", bufs=2,
                                       name=f"em_{sec}_{g}_{j}")
                    nc.vector.tensor_scalar_min(em[:], qe[:], 1.0)
                    if sec == 1 and is_halo:
                        kr = tmp_pool.tile([128, W], BF16, tag="kraw", bufs=2,
                                           name=f"kr_{g}_{j}")
                        nc.vector.scalar_tensor_tensor(
                            kr[:], ps[:], 0.0, em[:], AL.max, AL.add)
                        out_list[j] = qkv_pool.tile([128, W], BF16, tag=tag,
                                                    name=f"{tag}_{g}_{j}")
                        nc.gpsimd.tensor_scalar_mul(
                            out_list[j][:], kr[:], mask_s[:, 0:1])
                    else:
                        out_list[j] = qkv_pool.tile([128, W], BF16, tag=tag,
                                                    name=f"{tag}_{g}_{j}")
                        nc.vector.scalar_tensor_tensor(
                            out_list[j][:], ps[:], 0.0, em[:],
                            AL.max, AL.add)

        def emit_ksec_half(g, xts, js, k1):
            """k-section od tiles js: PE matmuls + phi drains."""
            W = WIDTHS[g]
            is_halo = g == 0
            for j in js:
                ps = ps_pool.tile([128, W], F32, tag="mm",
                                  name=f"pk_{g}_{j}")
                for k in range(NK):
                    nc.tensor.matmul(
                        ps[:], wq_sec[1][k][:, 128 * j:128 * (j + 1)],
                        xts[k][:], start=(k == 0), stop=(k == NK - 1))
                qe = tmp_pool.tile([128, W], BF16, tag="qe", bufs=2,
                                   name=f"qek_{g}_{j}")
                nc.scalar.activation(qe[:], ps[:], AF.Exp)
                em = tmp_pool.tile([128, W], BF16, tag="em", bufs=2,
                                   name=f"emk_{g}_{j}")
                nc.vector.tensor_scalar_min(em[:], qe[:], 1.0)
                if is_halo:
                    kr = tmp_pool.tile([128, W], BF16, tag="kraw", bufs=2,
                                       name=f"krk_{g}_{j}")
                    nc.vector.scalar_tensor_tensor(
                        kr[:], ps[:], 0.0, em[:], AL.max, AL.add)
                    k1[j] = qkv_pool.tile([128, W], BF16, tag="k10", bufs=8,
                                          name=f"k1_{g}_{j}")
                    nc.gpsimd.tensor_scalar_mul(
                        k1[j][:], kr[:], mask_s[:, 0:1])
                else:
                    k1[j] = qkv_pool.tile([128, W], BF16, tag="k1",
                                          name=f"k1_{g}_{j}")
                    nc.vector.scalar_tensor_tensor(
                        k1[j][:], ps[:], 0.0, em[:], AL.max, AL.add)

        def emit_oa(pg, p_qckv, p_deni):
            """bc broadcast matmuls + oa = qckv * (1/den)."""
            W = WIDTHS[pg]
            oa = [None] * NH
            for j in range(NH):
                bc = ps_pool.tile([128, W], F32, tag="mm", name=f"bc_{pg}_{j}")
                nc.tensor.matmul(
                    bc[:], bcsel_s[:, 128 * j:128 * (j + 1)], p_deni[:, :],
                    start=True, stop=True)
                oa[j] = oa_pool.tile([128, W], BF16, tag="oa",
                                     name=f"oa_{pg}_{j}")
                nc.vector.tensor_mul(oa[j][:], p_qckv[j][:], bc[:])
            return oa

        def emit_gate(pg, oa):
            W = WIDTHS[pg]
            gts = [None] * NH
            for ot in range(NH):
                ps = ps_pool.tile([128, W], F32, tag="mm", name=f"gp_{pg}_{ot}")
                for k in range(NK):
                    nc.tensor.matmul(
                        ps[:], wg_s[k][:, 128 * ot:128 * (ot + 1)], oa[k][:],
                        start=(k == 0), stop=(k == NK - 1))
                gts[ot] = gt_pool.tile([128, W], BF16, tag="gt",
                                       name=f"gt_{pg}_{ot}")
                nc.scalar.activation(
                    gts[ot][:], ps[:], AF.Sigmoid, bias=bgate_s[:, ot:ot + 1])
            return gts

        def emit_mix(pg, gts, oa, vv):
            # mix = g*oa + (1-g)*v = g*oa - (g-1)*v
            mix = [None] * NH
            for j in range(NH):
                m1 = tmp_pool.tile([128, WIDTHS[pg]], BF16, tag="m1", bufs=2,
                                   name=f"m1_{pg}_{j}")
                nc.vector.scalar_tensor_tensor(
                    m1[:], gts[j][:], -1.0, vv[j][:], AL.add, AL.mult)
                m2 = tmp_pool.tile([128, WIDTHS[pg]], BF16, tag="m2", bufs=2,
                                   name=f"m2_{pg}_{j}")
                nc.gpsimd.tensor_mul(m2[:], gts[j][:], oa[j][:])
                mix[j] = mix_pool.tile([128, WIDTHS[pg]], BF16, tag="mix",
                                       name=f"mix_{pg}_{j}")
                nc.vector.tensor_sub(mix[j][:], m2[:], m1[:])
            return mix

        def emit_y(pg, mix):
            W = WIDTHS[pg]
            out_tok = slice(OFFS[pg] - HALO, OFFS[pg] - HALO + W)
            for ot in range(NH):
                ps = ps_pool.tile([128, W], F32, tag="mm", name=f"yp_{pg}_{ot}")
                for k in range(NK):
                    nc.tensor.matmul(
                        ps[:], wo_s[k][:, 128 * ot:128 * (ot + 1)], mix[k][:],
                        start=(k == 0), stop=(k == NK - 1))
                ysb = y_pool.tile([128, W], BF16, tag="ysb",
                                  name=f"ysb_{pg}_{ot}")
                nc.scalar.copy(ysb[:], ps[:])
                nc.sync.dma_start(
                    yT.ap()[128 * ot:128 * (ot + 1), out_tok], ysb[:])

        def emit_ksum_scans(g, k1):
            W = WIDTHS[g]
            cum_ks = [None] * NH
            for j in range(NH):
                dec_b = dec_s[:, j:j + 1].broadcast_to([128, W])
                cum_ks[j] = cum_pool.tile([128, 512], BF16, tag=f"cks{j}",
                                          name=f"cks_{g}_{j}")
                init_ks = 0.0 if g == 0 else state["ks"][j][:, 0:1]
                nc.vector.tensor_tensor_scan(
                    cum_ks[j][:, :W], dec_b, k1[j][:], init_ks,
                    AL.mult, AL.add)
            nks = [None] * NH
            if g < NG - 1:
                for j in range(NH):
                    nks[j] = st_pool.tile([128, 1], F32, tag=f"sks{j}",
                                          name=f"sks_{g}_{j}")
                    nc.gpsimd.tensor_copy(nks[j][:], cum_ks[j][:, W - 1:W])
            state["ks"] = nks
            return cum_ks

        def emit_prods(g, q1, cum_ks):
            W = WIDTHS[g]
            prods = [None] * NH
            for j in range(NH):
                prods[j] = tmp_pool.tile([128, W], BF16, tag="prod",
                                         bufs=8, name=f"prod_{g}_{j}")
                nc.vector.tensor_mul(prods[j][:], q1[j][:], cum_ks[j][:, :W])
            return prods

        def emit_den(g, prods):
            W = WIDTHS[g]
            dps = psd_pool.tile([H, W], F32, tag="den", name=f"dps_{g}")
            for j in range(NH):
                nc.tensor.matmul(
                    dps[:], densel_s[:, H * j:H * (j + 1)], prods[j][:],
                    start=(j == 0), stop=(j == NH - 1))
            return dps

        def emit_recip(g, dps):
            W = WIDTHS[g]
            den_r = tmp_pool.tile([H, W], F32, tag="denr", name=f"denr_{g}")
            nc.vector.reciprocal_approx_fast(out=den_r[:], in_=dps[:])
            den_i = tmp_pool.tile([H, W], F32R, tag="deni", name=f"deni_{g}")
            with nc.allow_low_precision(reason="f32r broadcast of reciprocal"):
                nc.vector.tensor_copy(den_i[:], den_r[:])
            return den_i

        def emit_kvs(g, k1, vv):
            W = WIDTHS[g]
            kvs = [None] * NH
            for j in range(NH):
                kvs[j] = tmp_pool.tile([128, W], BF16,
                                       tag="kvp0" if g == 0 else "kvp",
                                       bufs=2,
                                       name=f"kv_{g}_{j}")
                nc.gpsimd.tensor_mul(kvs[j][:], k1[j][:], vv[j][:])
            return kvs

        def emit_kv_scans(g, kvs):
            W = WIDTHS[g]
            cum_kv = [None] * NH
            for j in range(NH):
                dec_b = dec_s[:, j:j + 1].broadcast_to([128, W])
                cum_kv[j] = cum_pool.tile([128, 512], BF16, tag=f"ckv{j}",
                                          name=f"ckv_{g}_{j}")
                init_kv = 0.0 if g == 0 else state["kv"][j][:, 0:1]
                nc.vector.tensor_tensor_scan(
                    cum_kv[j][:, :W], dec_b, kvs[j][:], init_kv,
                    AL.mult, AL.add)
            nkv = [None] * NH
            if g < NG - 1:
                for j in range(NH):
                    nkv[j] = st_pool.tile([128, 1], F32, tag=f"skv{j}",
                                          name=f"skv_{g}_{j}")
                    nc.gpsimd.tensor_copy(nkv[j][:], cum_kv[j][:, W - 1:W])
            state["kv"] = nkv
            return cum_kv

        def emit_qckv(g, q1, cum_kv):
            W = WIDTHS[g]
            qckv = [None] * NH
            for j in range(NH):
                qckv[j] = qkv_pool.tile([128, W], BF16, tag="qckv",
                                        name=f"qckv_{g}_{j}")
                nc.gpsimd.tensor_mul(qckv[j][:], q1[j][:], cum_kv[j][:, :W])
            return qckv

        # ---- prologue: halo group 0 interleaved with group 1 so the PE
        # tracks the weight-DMA arrival order (k-sec, v-sec, q-sec) ----
        xts0 = emit_xt(0)
        xts1 = emit_xt(1)
        k1_0 = [None] * NH
        emit_ksec_half(0, xts0, range(NH), k1_0)
        emit_ksum_scans(0, k1_0)
        k1_1 = [None] * NH
        emit_ksec_half(1, xts1, range(0, 4), k1_1)
        emit_ksec_half(1, xts1, range(4, NH), k1_1)
        cum_ks1 = emit_ksum_scans(1, k1_1)
        vv0 = [None] * NH
        emit_sec(0, xts0, 2, vv0, "v")
        kvs0 = emit_kvs(0, k1_0, vv0)
        emit_kv_scans(0, kvs0)
        q1_1 = [None] * NH
        emit_sec(1, xts1, 0, q1_1, "q1")
        prods1 = emit_prods(1, q1_1, cum_ks1)
        vv1 = [None] * NH
        emit_sec(1, xts1, 2, vv1, "v")
        kvs1 = emit_kvs(1, k1_1, vv1)
        dps1 = emit_den(1, prods1)
        den_i1 = emit_recip(1, dps1)
        cum_kv1 = emit_kv_scans(1, kvs1)
        qckv1 = emit_qckv(1, q1_1, cum_kv1)
        prev = (qckv1, vv1, den_i1, 1)
        xts_next = emit_xt(2)

        # ---- steady iterations g = 2..NG-1 ----
        # prev = (qckv, vv, den_i, pg) of the previous group
        for g in range(2, NG):
            xts = xts_next
            q1 = [None] * NH
            k1 = [None] * NH
            vv = [None] * NH
            emit_ksec_half(g, xts, range(0, 4), k1)
            p_qckv, p_vv, p_deni, pg = prev
            oa = emit_oa(pg, p_qckv, p_deni)
            emit_ksec_half(g, xts, range(4, NH), k1)
            cum_ks = emit_ksum_scans(g, k1)
            emit_sec(g, xts, 0, q1, "q1")
            if g < NG - 1:
                xts_next = emit_xt(g + 1)
            prods = emit_prods(g, q1, cum_ks)
            gts = emit_gate(pg, oa)
            dps = emit_den(g, prods)
            den_i = emit_recip(g, dps)
            mix = emit_mix(pg, gts, oa, p_vv)
            emit_sec(g, xts, 2, vv, "v")
            kvs = emit_kvs(g, k1, vv)
            cum_kv = emit_kv_scans(g, kvs)
            qckv = emit_qckv(g, q1, cum_kv)
            emit_y(pg, mix)
            prev = (qckv, vv, den_i, g)

        # ---- tail: attention output for the last group ----
        p_qckv, p_vv, p_deni, pg = prev
        oa = emit_oa(pg, p_qckv, p_deni)
        gts = emit_gate(pg, oa)
        mix = emit_mix(pg, gts, oa, p_vv)
        emit_y(pg, mix)

    nc.compile()
    return nc


def _sigmoid(v):
    return 1.0 / (1.0 + np.exp(-v))


def _make_inputs(x, Wqkv, Wout, Wgate, bgate, decay_param):
    decay = _sigmoid(np.asarray(decay_param, np.float64)).astype(np.float32)
    bf = ml_dtypes.bfloat16
    wqkvT = np.ascontiguousarray(np.asarray(Wqkv, np.float32).T).astype(bf)
    wgateT = np.ascontiguousarray(np.asarray(Wgate, np.float32).T).astype(bf)
    woutT = np.ascontiguousarray(np.asarray(Wout, np.float32).T).astype(bf)

    p = np.arange(128)
    dec_c = np.empty((128, NH), np.float32)
    for j in range(NH):
        dec_c[:, j] = decay[2 * j + p // 64]
    densel = np.zeros((128, NH * H), np.float32)
    for j in range(NH):
        for pp in range(128):
            densel[pp, H * j + 2 * j + pp // 64] = 1.0
    bcsel = np.zeros((H, NH * 128), np.float32)
    for j in range(NH):
        for m in range(128):
            bcsel[2 * j + m // 64, 128 * j + m] = 1.0
    bgate_c = np.ascontiguousarray(
        np.asarray(bgate, np.float32).reshape(NH, 128).T)

    in_maps = []
    for c in range(8):
        b, half = c // 2, c % 2
        xb = np.asarray(x[b], np.float32)  # [T, HID]
        if half == 0:
            xloc = np.concatenate(
                [np.zeros((HALO, HID), np.float32), xb[:HALF_T]], axis=0)
            mask = np.zeros((128, 1), np.float32)
        else:
            xloc = xb[HALF_T - HALO:]
            mask = np.ones((128, 1), np.float32)
        in_maps.append({
            "xT": np.ascontiguousarray(xloc.T).astype(bf),
            "wqkvT": wqkvT, "wgateT": wgateT, "woutT": woutT,
            "dec_c": dec_c, "mask_c": mask,
            "densel": densel.astype(bf), "bcsel": bcsel,
            "bgate_c": bgate_c,
        })
    return in_maps


def kernel(x, Wqkv, Wout, Wgate, bgate, decay_param):
    if "nc" not in _cache:
        _cache["nc"] = _build_nc()
    nc = _cache["nc"]
    in_maps = _make_inputs(x, Wqkv, Wout, Wgate, bgate, decay_param)
    res = run_bass_kernel_spmd(nc, in_maps, list(range(8)))
    y = np.empty((B, T, HID), np.float32)
    for c in range(8):
        b, half = c // 2, c % 2
        y[b, half * HALF_T:(half + 1) * HALF_T, :] = \
            res.results[c]["yT"].astype(np.float32).T
    return y


# revision 14
# speedup vs baseline: 1.1041x; 1.0194x over previous
"""Trainium2 Bass kernel for nn_LinearAttention (gated linear attention).

Math (per reference):
    qkv = x @ Wqkv.T ; q,k,v = split(qkv); q,k = elu(.)+1
    per (b,h): running_kv[t]  = d*running_kv[t-1]  + k[t]*v[t]   (elementwise, D=64)
               running_ksum[t]= d*running_ksum[t-1]+ k[t]
    den = clip(sum_d(q*running_ksum), 1e-6); out = q*running_kv/den
    g = sigmoid(out @ Wgate.T + bgate); out = g*out + (1-g)*v
    y = out @ Wout.T

Implementation strategy (8 NeuronCores, SPMD, no collectives):
  - Token-parallel: core c handles batch b=c//2, T-half h=c%2 (2048 tokens)
    plus a 128-token halo before the chunk to warm the decay scan
    (decay=0.95 => truncation error ~0.95^128 ~ 1.4e-3 relative, well under
    tolerance).  Half 0 gets a zero halo + k-mask so its state is exactly 0.
  - Layout: [feature(partition), token(free)]; host pre-transposes x and the
    weights so no on-chip transpose is needed; y comes out transposed.
  - Decay scans run on the Vector engine via tensor_tensor_scan, chained
    across token groups (g0=128-halo, then 4x512) via initial=state[:, -1:].
  - phi(x)=elu(x)+1 = min(exp(x),1) + relu(x): ACT Exp straight from PSUM,
    DVE tensor_scalar min (4x mode), DVE scalar_tensor_tensor (relu+add,
    PSUM src).  No PSUM->SBUF staging copy.
  - den: 0/1 block-diagonal selector matmul -> PSUM [16,512];
    reciprocal_approx_fast (custom DVE, ~5x faster than RECIPROCAL), cast to
    f32r, broadcast back to 128 partitions via selector matmul.
  - Steady-state per-engine schedule is software-pipelined so the PE never
    waits on the DVE FIFO:
      PE:  [k0-3 | bc(g-1) | k4-7 | q | gate(g-1) | den | v | y(g-1)]
      DVE: [phi-k | oa | phi-k | ksum-scans | phi-q | prods | recip |
            (g-1 mix) | kv-scans]
      ACT: [exp-k | exp-q | sigmoid | v-copy | y-copy]
      GPS: [ks-states | mix-mul | kvs | qckv | kv-states]
  - ~10 garbage matmuls on a memset tile at t=0 keep the PE HAM clock warm
    while the first weight DMAs land (spread over 4 DMA queues).
"""

import sys

for _p in ('/opt/trn_rl_repo', '/root/.axon_site'):
    if _p not in sys.path:
        sys.path.insert(0, _p)

from contextlib import ExitStack

import ml_dtypes
import numpy as np

import concourse.tile as tile
from concourse import bacc, mybir
from concourse.bass_utils import run_bass_kernel_spmd

F32 = mybir.dt.float32
F32R = mybir.dt.float32r
BF16 = mybir.dt.bfloat16
AL = mybir.AluOpType
AF = mybir.ActivationFunctionType

B, T, HID = 4, 4096, 1024
H, D = 16, 64
OD = 3 * HID              # 3072 qkv output rows
NK = HID // 128           # 8 hidden (contraction) tiles
HALF_T = T // 2           # 2048 tokens per core
HALO = 128
TLOC = HALO + HALF_T      # 2176
NH = HID // 128           # 8 tiles per q/k/v section
WIDTHS = [HALO, 512, 512, 512, 512]
OFFS = [0, 128, 640, 1152, 1664]
NG = len(WIDTHS)

_cache = {}


def _build_nc():
    nc = bacc.Bacc("TRN2", target_bir_lowering=False, debug=False)

    xT = nc.dram_tensor("xT", [HID, TLOC], BF16, kind="ExternalInput")
    wqkvT = nc.dram_tensor("wqkvT", [HID, OD], BF16, kind="ExternalInput")
    wgateT = nc.dram_tensor("wgateT", [HID, HID], BF16, kind="ExternalInput")
    woutT = nc.dram_tensor("woutT", [HID, HID], BF16, kind="ExternalInput")
    dec_c = nc.dram_tensor("dec_c", [128, NH], F32, kind="ExternalInput")
    mask_c = nc.dram_tensor("mask_c", [128, 1], F32, kind="ExternalInput")
    densel = nc.dram_tensor("densel", [128, NH * H], BF16, kind="ExternalInput")
    bcsel = nc.dram_tensor("bcsel", [H, NH * 128], F32R, kind="ExternalInput")
    bgate_c = nc.dram_tensor("bgate_c", [128, NH], F32, kind="ExternalInput")
    yT = nc.dram_tensor("yT", [HID, HALF_T], BF16, kind="ExternalOutput")

    with tile.TileContext(nc) as tc, ExitStack() as ctx:
        consts = ctx.enter_context(tc.tile_pool(name="consts", bufs=1))
        wq_pool = ctx.enter_context(tc.tile_pool(name="wq", bufs=1))
        wg_pool = ctx.enter_context(tc.tile_pool(name="wgp", bufs=1))
        wo_pool = ctx.enter_context(tc.tile_pool(name="wop", bufs=1))
        xt_pool = ctx.enter_context(tc.tile_pool(name="xt", bufs=10))
        qkv_pool = ctx.enter_context(tc.tile_pool(name="qkv", bufs=9))
        tmp_pool = ctx.enter_context(tc.tile_pool(name="tmp", bufs=2))
        cum_pool = ctx.enter_context(tc.tile_pool(name="cum", bufs=1))
        st_pool = ctx.enter_context(tc.tile_pool(name="st", bufs=2))
        oa_pool = ctx.enter_context(tc.tile_pool(name="oa", bufs=8))
        gt_pool = ctx.enter_context(tc.tile_pool(name="gt", bufs=8))
        mix_pool = ctx.enter_context(tc.tile_pool(name="mix", bufs=8))
        y_pool = ctx.enter_context(tc.tile_pool(name="ysb", bufs=3))
        ps_pool = ctx.enter_context(tc.tile_pool(name="ps", bufs=7, space="PSUM"))
        psd_pool = ctx.enter_context(tc.tile_pool(name="psd", bufs=1, space="PSUM"))

        # ---- warmup: keep the PE HAM clock busy while weight DMAs land ----
        warm = consts.tile([128, 256], BF16, tag="warm")
        nc.gpsimd.memset(warm[:], 0.25)
        wps = ps_pool.tile([128, 512], F32, tag="mm", name="warm_ps")
        for i in range(28):
            nc.tensor.matmul(wps[:, 0:256], warm[:, 0:128], warm[:],
                             start=True, stop=True)

        # ---- small consts (gpsimd queue) ----
        dec_s = consts.tile([128, NH], F32, tag="dec")
        nc.gpsimd.dma_start(dec_s[:], dec_c.ap()[:, :])
        mask_s = consts.tile([128, 1], F32, tag="mask")
        nc.gpsimd.dma_start(mask_s[:], mask_c.ap()[:, :])
        densel_s = consts.tile([128, NH * H], BF16, tag="densel")
        nc.gpsimd.dma_start(densel_s[:], densel.ap()[:, :])
        bcsel_s = consts.tile([H, NH * 128], F32R, tag="bcsel")
        nc.gpsimd.dma_start(bcsel_s[:], bcsel.ap()[:, :])
        bgate_s = consts.tile([128, NH], F32, tag="bg")
        nc.gpsimd.dma_start(bgate_s[:], bgate_c.ap()[:, :])

        # ---- qkv weights: interleave k-tiles across the act/vector queues
        # in consumption order: k-section, v-section, q-section ----
        wq_sec = {}
        for sec in range(3):
            wq_sec[sec] = [
                wq_pool.tile([128, HID], BF16, tag=f"wq{sec}_{k}",
                             name=f"wq_{sec}_{k}")
                for k in range(NK)]
        wg_s, wo_s = [], []
        for k in range(NK):
            wg_s.append(wg_pool.tile([128, HID], BF16, tag=f"wg{k}",
                                     name=f"wg_{k}"))
            wo_s.append(wo_pool.tile([128, HID], BF16, tag=f"wo{k}",
                                     name=f"wo_{k}"))

        _rings = [nc.scalar, nc.gpsimd, nc.sync]

        def load_wq_sec(sec):
            for k in range(NK):
                _rings[k % 3].dma_start(
                    wq_sec[sec][k][:],
                    wqkvT.ap()[128 * k:128 * (k + 1), HID * sec:HID * (sec + 1)])

        def load_gate_out():
            for k in range(NK):
                _rings[k % 3].dma_start(
                    wg_s[k][:], wgateT.ap()[128 * k:128 * (k + 1), :])
            for k in range(NK):
                _rings[k % 3].dma_start(
                    wo_s[k][:], woutT.ap()[128 * k:128 * (k + 1), :])

        state = {}

        def emit_xt(g):
            W = WIDTHS[g]
            tok = slice(OFFS[g], OFFS[g] + W)
            xts = []
            for k in range(NK):
                xt_t = xt_pool.tile([128, W], BF16,
                                    tag="xt0" if g == 0 else "xt",
                                    bufs=8 if g == 0 else 10,
                                    name=f"xt_{g}_{k}")
                nc.sync.dma_start(xt_t[:], xT.ap()[128 * k:128 * (k + 1), tok])
                xts.append(xt_t)
            return xts

        def emit_sec(g, xts, sec, out_list, tag):
            """One qkv section (8 od tiles): PE matmuls + phi/copy drains.
            js selects which od tiles of the section to emit."""
            W = WIDTHS[g]
            is_halo = g == 0
            for j in range(NH):
                ps = ps_pool.tile([128, W], F32, tag="mm",
                                  name=f"p{sec}_{g}_{j}")
                for k in range(NK):
                    nc.tensor.matmul(
                        ps[:], wq_sec[sec][k][:, 128 * j:128 * (j + 1)],
                        xts[k][:], start=(k == 0), stop=(k == NK - 1))
                if sec == 2:  # v: plain copy
                    out_list[j] = qkv_pool.tile(
                        [128, W], BF16, tag="v0" if is_halo else "v",
                        bufs=8 if is_halo else 9,
                        name=f"v_{g}_{j}")
                    nc.scalar.copy(out_list[j][:], ps[:])
                else:
                    qe = tmp_pool.tile([128, W], BF16, tag="qe", bufs=2,
                                       name=f"qe_{sec}_{g}_{j}")
                    nc.scalar.activation(qe[:], ps[:], AF.Exp)
                    em = tmp_pool.tile([128, W], BF16, tag="em", bufs=2,
                                       name=f"em_{sec}_{g}_{j}")
                    nc.vector.tensor_scalar_min(em[:], qe[:], 1.0)
                    if sec == 1 and is_halo:
                        kr = tmp_pool.tile([128, W], BF16, tag="kraw", bufs=2,
                                           name=f"kr_{g}_{j}")
                        nc.vector.scalar_tensor_tensor(
                            kr[:], ps[:], 0.0, em[:], AL.max, AL.add)
                        out_list[j] = qkv_pool.tile([128, W], BF16, tag=tag,
                                                    name=f"{tag}_{g}_{j}")
                        nc.gpsimd.tensor_scalar_mul(
                            out_list[j][:], kr[:], mask_s[:, 0:1])
                    else:
                        out_list[j] = qkv_pool.tile([128, W], BF16, tag=tag,
                                                    name=f"{tag}_{g}_{j}")
                        nc.vector.scalar_tensor_tensor(
                            out_list[j][:], ps[:], 0.0, em[:],
                            AL.max, AL.add)

        def emit_ksec_half(g, xts, js, k1):
            """k-section od tiles js: PE matmuls + phi drains."""
            W = WIDTHS[g]
            is_halo = g == 0
            for j in js:
                ps = ps_pool.tile([128, W], F32, tag="mm",
                                  name=f"pk_{g}_{j}")
                for k in range(NK):
                    nc.tensor.matmul(
                        ps[:], wq_sec[1][k][:, 128 * j:128 * (j + 1)],
                        xts[k][:], start=(k == 0), stop=(k == NK - 1))
                qe = tmp_pool.tile([128, W], BF16, tag="qe", bufs=2,
                                   name=f"qek_{g}_{j}")
                nc.scalar.activation(qe[:], ps[:], AF.Exp)
                em = tmp_pool.tile([128, W], BF16, tag="em", bufs=2,
                                   name=f"emk_{g}_{j}")
                nc.vector.tensor_scalar_min(em[:], qe[:], 1.0)
                if is_halo:
                    kr = tmp_pool.tile([128, W], BF16, tag="kraw", bufs=2,
                                       name=f"krk_{g}_{j}")
                    nc.vector.scalar_tensor_tensor(
                        kr[:], ps[:], 0.0, em[:], AL.max, AL.add)
                    k1[j] = qkv_pool.tile([128, W], BF16, tag="k10", bufs=8,
                                          name=f"k1_{g}_{j}")
                    nc.gpsimd.tensor_scalar_mul(
                        k1[j][:], kr[:], mask_s[:, 0:1])
                else:
                    k1[j] = qkv_pool.tile([128, W], BF16, tag="k1",
                                          name=f"k1_{g}_{j}")
                    nc.vector.scalar_tensor_tensor(
                        k1[j][:], ps[:], 0.0, em[:], AL.max, AL.add)

        def emit_oa(pg, p_qckv, p_deni):
            """bc broadcast matmuls + oa = qckv * (1/den)."""
            W = WIDTHS[pg]
            oa = [None] * NH
            for j in range(NH):
                bc = ps_pool.tile([128, W], F32, tag="mm", name=f"bc_{pg}_{j}")
                nc.tensor.matmul(
                    bc[:], bcsel_s[:, 128 * j:128 * (j + 1)], p_deni[:, :],
                    start=True, stop=True)
                oa[j] = oa_pool.tile([128, W], BF16, tag="oa",
                                     name=f"oa_{pg}_{j}")
                nc.vector.tensor_mul(oa[j][:], p_qckv[j][:], bc[:])
            return oa

        def emit_gate(pg, oa):
            W = WIDTHS[pg]
            gts = [None] * NH
            for ot in range(NH):
                ps = ps_pool.tile([128, W], F32, tag="mm", name=f"gp_{pg}_{ot}")
                for k in range(NK):
                    nc.tensor.matmul(
                        ps[:], wg_s[k][:, 128 * ot:128 * (ot + 1)], oa[k][:],
                        start=(k == 0), stop=(k == NK - 1))
                gts[ot] = gt_pool.tile([128, W], BF16, tag="gt",
                                       name=f"gt_{pg}_{ot}")
                nc.scalar.activation(
                    gts[ot][:], ps[:], AF.Sigmoid, bias=bgate_s[:, ot:ot + 1])
            return gts

        def emit_mix(pg, gts, oa, vv):
            # mix = g*oa + (1-g)*v = g*oa - (g-1)*v
            mix = [None] * NH
            for j in range(NH):
                m1 = tmp_pool.tile([128, WIDTHS[pg]], BF16, tag="m1", bufs=2,
                                   name=f"m1_{pg}_{j}")
                nc.vector.scalar_tensor_tensor(
                    m1[:], gts[j][:], -1.0, vv[j][:], AL.add, AL.mult)
                m2 = tmp_pool.tile([128, WIDTHS[pg]], BF16, tag="m2", bufs=2,
                                   name=f"m2_{pg}_{j}")
                nc.gpsimd.tensor_mul(m2[:], gts[j][:], oa[j][:])
                mix[j] = mix_pool.tile([128, WIDTHS[pg]], BF16, tag="mix",
                                       name=f"mix_{pg}_{j}")
                nc.vector.tensor_sub(mix[j][:], m2[:], m1[:])
            return mix

        def emit_y(pg, mix):
            W = WIDTHS[pg]
            out_tok = slice(OFFS[pg] - HALO, OFFS[pg] - HALO + W)
            for ot in range(NH):
                ps = ps_pool.tile([128, W], F32, tag="mm", name=f"yp_{pg}_{ot}")
                for k in range(NK):
                    nc.tensor.matmul(
                        ps[:], wo_s[k][:, 128 * ot:128 * (ot + 1)], mix[k][:],
                        start=(k == 0), stop=(k == NK - 1))
                ysb = y_pool.tile([128, W], BF16, tag="ysb",
                                  name=f"ysb_{pg}_{ot}")
                nc.scalar.copy(ysb[:], ps[:])
                nc.sync.dma_start(
                    yT.ap()[128 * ot:128 * (ot + 1), out_tok], ysb[:])

        def emit_ksum_scans(g, k1):
            W = WIDTHS[g]
            cum_ks = [None] * NH
            for j in range(NH):
                dec_b = dec_s[:, j:j + 1].broadcast_to([128, W])
                cum_ks[j] = cum_pool.tile([128, 512], BF16, tag=f"cks{j}",
                                          name=f"cks_{g}_{j}")
                init_ks = 0.0 if g == 0 else state["ks"][j][:, 0:1]
                nc.vector.tensor_tensor_scan(
                    cum_ks[j][:, :W], dec_b, k1[j][:], init_ks,
                    AL.mult, AL.add)
            nks = [None] * NH
            if g < NG - 1:
                for j in range(NH):
                    nks[j] = st_pool.tile([128, 1], F32, tag=f"sks{j}",
                                          name=f"sks_{g}_{j}")
                    nc.gpsimd.tensor_copy(nks[j][:], cum_ks[j][:, W - 1:W])
            state["ks"] = nks
            return cum_ks

        def emit_prods(g, q1, cum_ks):
            W = WIDTHS[g]
            prods = [None] * NH
            for j in range(NH):
                prods[j] = tmp_pool.tile([128, W], BF16, tag="prod",
                                         bufs=8, name=f"prod_{g}_{j}")
                nc.vector.tensor_mul(prods[j][:], q1[j][:], cum_ks[j][:, :W])
            return prods

        def emit_den(g, prods):
            W = WIDTHS[g]
            dps = psd_pool.tile([H, W], F32, tag="den", name=f"dps_{g}")
            for j in range(NH):
                nc.tensor.matmul(
                    dps[:], densel_s[:, H * j:H * (j + 1)], prods[j][:],
                    start=(j == 0), stop=(j == NH - 1))
            return dps

        def emit_recip(g, dps):
            W = WIDTHS[g]
            den_r = tmp_pool.tile([H, W], F32, tag="denr", name=f"denr_{g}")
            nc.vector.reciprocal_approx_fast(out=den_r[:], in_=dps[:])
            den_i = tmp_pool.tile([H, W], F32R, tag="deni", name=f"deni_{g}")
            with nc.allow_low_precision(reason="f32r broadcast of reciprocal"):
                nc.vector.tensor_copy(den_i[:], den_r[:])
            return den_i

        def emit_kvs(g, k1, vv):
            W = WIDTHS[g]
            kvs = [None] * NH
            for j in range(NH):
                kvs[j] = tmp_pool.tile([128, W], BF16,
                                       tag="kvp0" if g == 0 else "kvp",
                                       bufs=2,
                                       name=f"kv_{g}_{j}")
                nc.gpsimd.tensor_mul(kvs[j][:], k1[j][:], vv[j][:])
            return kvs

        def emit_kv_scans(g, kvs):
            W = WIDTHS[g]
            cum_kv = [None] * NH
            for j in range(NH):
                dec_b = dec_s[:, j:j + 1].broadcast_to([128, W])
                cum_kv[j] = cum_pool.tile([128, 512], BF16, tag=f"ckv{j}",
                                          name=f"ckv_{g}_{j}")
                init_kv = 0.0 if g == 0 else state["kv"][j][:, 0:1]
                nc.vector.tensor_tensor_scan(
                    cum_kv[j][:, :W], dec_b, kvs[j][:], init_kv,
                    AL.mult, AL.add)
            nkv = [None] * NH
            if g < NG - 1:
                for j in range(NH):
                    nkv[j] = st_pool.tile([128, 1], F32, tag=f"skv{j}",
                                          name=f"skv_{g}_{j}")
                    nc.gpsimd.tensor_copy(nkv[j][:], cum_kv[j][:, W - 1:W])
            state["kv"] = nkv
            return cum_kv

        def emit_qckv(g, q1, cum_kv):
            W = WIDTHS[g]
            qckv = [None] * NH
            for j in range(NH):
                qckv[j] = qkv_pool.tile([128, W], BF16, tag="qckv",
                                        name=f"qckv_{g}_{j}")
                nc.gpsimd.tensor_mul(qckv[j][:], q1[j][:], cum_kv[j][:, :W])
            return qckv

        # ---- prologue: halo group 0 interleaved with group 1, emitted in
        # weight-DMA arrival order (k-sec, v-sec, q-sec); xt triggers go
        # first on the SP ring, ahead of its weight share ----
        xts0 = emit_xt(0)
        xts1 = emit_xt(1)
        load_wq_sec(1)
        load_wq_sec(2)
        k1_0 = [None] * NH
        emit_ksec_half(0, xts0, range(NH), k1_0)
        emit_ksum_scans(0, k1_0)
        k1_1 = [None] * NH
        emit_ksec_half(1, xts1, range(0, 4), k1_1)
        emit_ksec_half(1, xts1, range(4, NH), k1_1)
        cum_ks1 = emit_ksum_scans(1, k1_1)
        load_wq_sec(0)
        vv0 = [None] * NH
        emit_sec(0, xts0, 2, vv0, "v")
        kvs0 = emit_kvs(0, k1_0, vv0)
        emit_kv_scans(0, kvs0)
        vv1 = [None] * NH
        emit_sec(1, xts1, 2, vv1, "v")
        kvs1 = emit_kvs(1, k1_1, vv1)
        cum_kv1 = emit_kv_scans(1, kvs1)
        xts_next = emit_xt(2)
        load_gate_out()
        q1_1 = [None] * NH
        emit_sec(1, xts1, 0, q1_1, "q1")
        prods1 = emit_prods(1, q1_1, cum_ks1)
        dps1 = emit_den(1, prods1)
        den_i1 = emit_recip(1, dps1)
        qckv1 = emit_qckv(1, q1_1, cum_kv1)
        prev = (qckv1, vv1, den_i1, 1)

        # ---- steady iterations g = 2..NG-1 ----
        # prev = (qckv, vv, den_i, pg) of the previous group
        for g in range(2, NG):
            xts = xts_next
            q1 = [None] * NH
            k1 = [None] * NH
            vv = [None] * NH
            emit_ksec_half(g, xts, range(0, 4), k1)
            p_qckv, p_vv, p_deni, pg = prev
            oa = emit_oa(pg, p_qckv, p_deni)
            emit_ksec_half(g, xts, range(4, NH), k1)
            cum_ks = emit_ksum_scans(g, k1)
            emit_sec(g, xts, 0, q1, "q1")
            if g < NG - 1:
                xts_next = emit_xt(g + 1)
            prods = emit_prods(g, q1, cum_ks)
            gts = emit_gate(pg, oa)
            dps = emit_den(g, prods)
            den_i = emit_recip(g, dps)
            mix = emit_mix(pg, gts, oa, p_vv)
            emit_sec(g, xts, 2, vv, "v")
            kvs = emit_kvs(g, k1, vv)
            cum_kv = emit_kv_scans(g, kvs)
            qckv = emit_qckv(g, q1, cum_kv)
            emit_y(pg, mix)
            prev = (qckv, vv, den_i, g)

        # ---- tail: attention output for the last group ----
        p_qckv, p_vv, p_deni, pg = prev
        oa = emit_oa(pg, p_qckv, p_deni)
        gts = emit_gate(pg, oa)
        mix = emit_mix(pg, gts, oa, p_vv)
        emit_y(pg, mix)

    nc.compile()
    return nc


def _sigmoid(v):
    return 1.0 / (1.0 + np.exp(-v))


def _make_inputs(x, Wqkv, Wout, Wgate, bgate, decay_param):
    decay = _sigmoid(np.asarray(decay_param, np.float64)).astype(np.float32)
    bf = ml_dtypes.bfloat16
    wqkvT = np.ascontiguousarray(np.asarray(Wqkv, np.float32).T).astype(bf)
    wgateT = np.ascontiguousarray(np.asarray(Wgate, np.float32).T).astype(bf)
    woutT = np.ascontiguousarray(np.asarray(Wout, np.float32).T).astype(bf)

    p = np.arange(128)
    dec_c = np.empty((128, NH), np.float32)
    for j in range(NH):
        dec_c[:, j] = decay[2 * j + p // 64]
    densel = np.zeros((128, NH * H), np.float32)
    for j in range(NH):
        for pp in range(128):
            densel[pp, H * j + 2 * j + pp // 64] = 1.0
    bcsel = np.zeros((H, NH * 128), np.float32)
    for j in range(NH):
        for m in range(128):
            bcsel[2 * j + m // 64, 128 * j + m] = 1.0
    bgate_c = np.ascontiguousarray(
        np.asarray(bgate, np.float32).reshape(NH, 128).T)

    in_maps = []
    for c in range(8):
        b, half = c // 2, c % 2
        xb = np.asarray(x[b], np.float32)  # [T, HID]
        if half == 0:
            xloc = np.concatenate(
                [np.zeros((HALO, HID), np.float32), xb[:HALF_T]], axis=0)
            mask = np.zeros((128, 1), np.float32)
        else:
            xloc = xb[HALF_T - HALO:]
            mask = np.ones((128, 1), np.float32)
        in_maps.append({
            "xT": np.ascontiguousarray(xloc.T).astype(bf),
            "wqkvT": wqkvT, "wgateT": wgateT, "woutT": woutT,
            "dec_c": dec_c, "mask_c": mask,
            "densel": densel.astype(bf), "bcsel": bcsel,
            "bgate_c": bgate_c,
        })
    return in_maps


def kernel(x, Wqkv, Wout, Wgate, bgate, decay_param):
    if "nc" not in _cache:
        _cache["nc"] = _build_nc()
    nc = _cache["nc"]
    in_maps = _make_inputs(x, Wqkv, Wout, Wgate, bgate, decay_param)
    res = run_bass_kernel_spmd(nc, in_maps, list(range(8)))
    y = np.empty((B, T, HID), np.float32)
    for c in range(8):
        b, half = c // 2, c % 2
        y[b, half * HALF_T:(half + 1) * HALF_T, :] = \
            res.results[c]["yT"].astype(np.float32).T
    return y
